# revision 30
# baseline (speedup 1.0000x reference)
"""Distributed ISTFT kernel for Trainium2 (8 NeuronCores, Bass/Tile).

Math (matches the jax reference):
  z: [2, 513, T] one-sided spectrum (real/imag), T = 8192 frames.
  Hermitian extension + ifft(1024) + window + overlap-add (hop 256) +
  divide by overlapped window sum + trim 512 each side -> [2, 2096896].

Key folds used here:
  * real(ifft) = A^T @ X where A [1024(k), 1024(n)] packs the cos rows for
    zr bins 0..512 and sin rows for zi bins 1..511; X packs those z rows.
  * imag(ifft)[n, t] = (zi[0,t] + (-1)^n zi[512,t]) / N  (rank-2).
  * Output sample m = 256*b + r; block b = sum_{q=0..3} wf_{b-q}[256q+r].
    Folding window * A, and the interior 1/window-sum, into the stationary
    operand on the HOST gives O^T[t, r] = sum_q X[:, t+3-q]^T @ Aw_q
    directly -- overlap-add, windowing and normalization all ride inside
    the matmul and the device never touches the window.
  * Frame axis is sharded 1024 output blocks/core with a 3-frame input
    halo, so no cross-core communication is needed at all.  The two
    blocks whose window-sum misses a frame (first and last output block)
    get a row fixup whose factor is a per-core host-computed input
    (1.0 on interior cores), so all cores run the same program.
  * Everything the PE touches is bf16 (tolerance is 2e-2; bf16 ends up
    ~3e-3): input DMA halves vs fp32, and the moving operand still
    streams at 1 column/cycle.  Output is written bf16 and widened on
    the host.
  * DMA descriptors are issued inline by the engines at ~600ns per
    dma_start, so X and Aw are stored chunk-column-major in DRAM by the
    host and fetched in 4 grouped, fully contiguous DMAs each; output
    tiles are evicted in pairs (one DMA per 256 blocks).  A short burst
    of throwaway matmuls at t=0 warms the PE's HAM clock gate
    (1.2 -> 2.4 GHz) while the first real DMAs land.
"""

import numpy as np
import ml_dtypes

N_FFT = 1024
HOP = 256
T_FRAMES = 8192
N_CORES = 8
F_SLOTS = 1027  # frame slots per core: 1024 owned blocks need slots t..t+3
NB = 1024       # output blocks computed per core (core 7 uses 1023)
N_WARM = 17     # PE warmup matmuls issued while first DMAs are in flight

_CACHE = {}


def _amat() -> np.ndarray:
    """A [1024(kappa), 1024(n)]: ifft cos/sin weights, f64."""
    n = np.arange(N_FFT, dtype=np.float64)[None, :]
    k = np.arange(513, dtype=np.float64)[:, None]
    g = np.full((513, 1), 2.0)
    g[0, 0] = 1.0
    g[512, 0] = 1.0
    C = (g / N_FFT) * np.cos(2.0 * np.pi * k * n / N_FFT)
    k2 = np.arange(1, 512, dtype=np.float64)[:, None]
    S = (-2.0 / N_FFT) * np.sin(2.0 * np.pi * k2 * n / N_FFT)
    return np.concatenate([C, S], 0)


def _build_nc():
    from contextlib import ExitStack

    import concourse.tile as tile
    from concourse import bacc, mybir

    f32 = mybir.dt.float32
    bf = mybir.dt.bfloat16

    nc = bacc.Bacc("TRN2", target_bir_lowering=False, debug=False,
                   num_devices=N_CORES)

    # x and Aw are chunk-column-major: element [p, 1027*k + s] holds
    # X[128*k + p, s], so every DMA below is a contiguous column span.
    x_d = nc.dram_tensor("x", [128, 8 * F_SLOTS], bf, kind="ExternalInput")
    z_d = nc.dram_tensor("zrows", [2, F_SLOTS], bf, kind="ExternalInput")
    a_d = nc.dram_tensor("awm", [128, 8 * N_FFT], bf, kind="ExternalInput")
    t_d = nc.dram_tensor("taps", [8, 256], bf, kind="ExternalInput")
    f_d = nc.dram_tensor("fix", [2, 256], f32, kind="ExternalInput")
    o_d = nc.dram_tensor("out", [2, NB, 256], bf, kind="ExternalOutput")

    with tile.TileContext(nc) as tc, ExitStack() as ctx:
        big = ctx.enter_context(tc.tile_pool(name="big", bufs=1))
        sml = ctx.enter_context(tc.tile_pool(name="sml", bufs=1))
        # bank budget: 6 (ps0) + 2 (transient warmup psum, then ps1) = 8
        ps0p = ctx.enter_context(tc.tile_pool(name="ps0p", bufs=6, space="PSUM"))
        osb = ctx.enter_context(tc.tile_pool(name="osb", bufs=8))

        # ---- PE warmup: dep-free matmuls on a memset tile keep the PE
        # busy from the end of the engine preamble so the HAM clock gate
        # releases (1.2 -> 2.4 GHz) while the first real DMAs land.
        wrm = sml.tile([128, 256], bf, tag="wrm")
        nc.vector.memset(wrm[:], 0.0)
        with tc.tile_pool(name="psw", bufs=1, space="PSUM") as psw:
            ps_w = psw.tile([128, 256], f32, tag="ps_warm")
            for _ in range(N_WARM):
                nc.tensor.matmul(ps_w[:], lhsT=wrm[:, 0:128], rhs=wrm[:],
                                 start=True, stop=True)
        ps1p = ctx.enter_context(tc.tile_pool(name="ps1p", bufs=2, space="PSUM"))

        # ---- big streams first, one DMA per chunk (x on the sync
        # queue, Aw on the scalar queue -- HBM is the shared limit, and
        # each queue drains FIFO in consumption order).  The first Aw
        # chunk is needed whole by matmul #1, but x chunk 0 is consumed
        # two tt-blocks at a time, so it is split into four
        # column-range tiles (262 cols each, 3-col overlap) that
        # unblock the first matmuls as soon as ~67KB have landed.
        aw_t = [None] * 8
        xs_t = [None] * 8
        for k in range(8):
            xg = big.tile([128, F_SLOTS], bf, tag=f"xg{k}", name=f"xg{k}")
            nc.sync.dma_start(
                out=xg[:], in_=x_d.ap()[:, k * F_SLOTS:(k + 1) * F_SLOTS])
            xs_t[k] = (xg, 0)
            ag = big.tile([128, N_FFT], bf, tag=f"ag{k}", name=f"ag{k}")
            nc.scalar.dma_start(
                out=ag[:], in_=a_d.ap()[:, k * N_FFT:(k + 1) * N_FFT])
            aw_t[k] = (ag, 0)

        # ---- tiny setup inputs on the gpsimd queue.  The imag-channel
        # operands are padded to the full 128 partitions with zeros so
        # the channel-1 matmuls are ordinary full-array matmuls (a
        # row-group-restricted matmul costs a ~200ns pipeline bubble
        # on entry and exit).
        taps = sml.tile([128, 256], bf, tag="taps")
        nc.vector.memset(taps[:], 0.0)
        nc.gpsimd.dma_start(out=taps[0:8, :], in_=t_d.ap())
        # partitions 0-3 = zi0 shifted by q, 4-7 = zi512 shifted, rest 0
        tuv = sml.tile([128, NB], bf, tag="tuv")
        nc.vector.memset(tuv[:], 0.0)
        for q in range(4):
            nc.gpsimd.dma_start(out=tuv[q:q + 1, :],
                                in_=z_d.ap()[0:1, 3 - q:3 - q + NB])
            nc.gpsimd.dma_start(out=tuv[4 + q:5 + q, :],
                                in_=z_d.ap()[1:2, 3 - q:3 - q + NB])
        # full-height fixup factor tiles: all-ones except the one special
        # row (engines can't address a lone partition 126; DMA can)
        fxa = sml.tile([128, 256], f32, tag="fxa")
        nc.vector.memset(fxa[:], 1.0)
        nc.gpsimd.dma_start(out=fxa[0:1, :], in_=f_d.ap()[0:1, :])
        fxb = sml.tile([128, 256], f32, tag="fxb")
        nc.vector.memset(fxb[:], 1.0)
        nc.gpsimd.dma_start(out=fxb[126:127, :], in_=f_d.ap()[1:2, :])

        def evict_half(o, half, ps, tt):
            cols = slice(256 * half, 256 * (half + 1))
            if tt == 0:
                nc.vector.tensor_mul(o[:, cols], ps[:], fxa[:])
            elif tt == 7:
                nc.vector.tensor_mul(o[:, cols], ps[:], fxb[:])
            else:
                nc.vector.tensor_copy(o[:, cols], ps[:])

        def evict_pair(ps_a, ta, ps_b, tb, ch):
            # two adjacent tt tiles -> one [128, 512] sbuf tile -> one DMA
            # covering out[ch, ta*128:(ta+2)*128, :]
            o = osb.tile([128, 512], bf, tag=f"o{ch}", name=f"o{ch}_{ta}")
            evict_half(o, 0, ps_a, ta)
            evict_half(o, 1, ps_b, tb)
            dst = o_d.ap()[ch:ch + 1, ta * 128:(tb + 1) * 128, :]
            nc.sync.dma_start(
                out=dst.rearrange("a (h p) r -> a p h r", p=128), in_=o[:])

        def evict_one(ps, tt, ch):
            o = osb.tile([128, 256], bf, tag=f"s{ch}", name=f"s{ch}_{tt}")
            evict_half(o, 0, ps, tt)
            nc.sync.dma_start(
                out=o_d.ap()[ch:ch + 1, tt * 128:(tt + 1) * 128, :], in_=o[:])
            return o

        def ch1_pair(ta):
            # both imag-channel blocks of a pair back-to-back (one
            # row-group switch), then one paired eviction DMA
            ps = []
            for tt in (ta, ta + 1):
                p1 = ps1p.tile([128, 256], f32, tag="ps1", name=f"ps1_{tt}")
                nc.tensor.matmul(p1[:], lhsT=tuv[:, tt * 128:tt * 128 + 128],
                                 rhs=taps[:], start=True, stop=True)
                ps.append(p1)
            evict_pair(ps[0], ta, ps[1], ta + 1, 1)

        def mm_block(ps, tt, k, qs):
            xg, xb = xs_t[k]
            ag, ab = aw_t[k]
            for q in qs:
                off = xb + tt * 128 + 3 - q
                nc.tensor.matmul(
                    ps[:],
                    lhsT=xg[:, off:off + 128],
                    rhs=ag[:, ab + 256 * q:ab + 256 * (q + 1)],
                    start=(k == 0 and q == 0),
                    stop=(k == 7 and q == 3))

        # ---- channel 0 sweep 1: k-outer accumulation over six psum
        # tiles, following the arriving k chunks; paired evictions
        pss = {tt: ps0p.tile([128, 256], f32, tag="ps0", name=f"ps0_{tt}")
               for tt in range(6)}
        for k in range(8):
            for tt in range(6):
                mm_block(pss[tt], tt, k, range(4))
        for ta in (0, 2, 4):
            evict_pair(pss[ta], ta, pss[ta + 1], ta + 1, 0)

        # ---- sweep 2: tt-outer (all inputs are resident by now) so the
        # tail holds only one eviction; channel-1 pairs slot between
        # half-blocks, away from the very end
        ps6 = ps0p.tile([128, 256], f32, tag="ps0", name="ps0_6")
        for k in range(4):
            mm_block(ps6, 6, k, range(4))
        ch1_pair(0)
        for k in range(4, 8):
            mm_block(ps6, 6, k, range(4))
        ch1_pair(2)
        s06 = evict_one(ps6, 6, 0)
        ps7 = ps0p.tile([128, 256], f32, tag="ps0", name="ps0_7")
        for k in range(2):
            mm_block(ps7, 7, k, range(4))
        ch1_pair(4)
        for k in range(2, 4):
            mm_block(ps7, 7, k, range(4))
        ch1_pair(6)
        for k in range(4, 7):
            mm_block(ps7, 7, k, range(4))
        # re-write 4KB of the already-evicted tt6 tile to keep the sync
        # DMA queue's engines hot, so the final eviction's DMA doesn't
        # pay a cold-queue kick right on the critical tail
        nc.sync.dma_start(
            out=o_d.ap()[0:1, 6 * 128:6 * 128 + 8, :], in_=s06[0:8, :])
        mm_block(ps7, 7, 7, range(4))
        evict_one(ps7, 7, 0)

    nc.compile()
    return nc


def _host_consts(window: np.ndarray):
    """Window-dependent host folds: Aw matrix, ch1 taps, edge fixups."""
    w = np.asarray(window, np.float64)
    ws4 = w[0:256] + w[256:512] + w[512:768] + w[768:1024]
    # reference guard: only divide where the overlapped window sum >= 1e-6
    d4 = np.where(ws4 >= 1e-6, ws4, 1.0)
    d3a = np.where(ws4 - w[768:1024] >= 1e-6, ws4 - w[768:1024], 1.0)
    d3b = np.where(ws4 - w[0:256] >= 1e-6, ws4 - w[0:256], 1.0)

    colscale = w / np.tile(d4, 4)                   # w[n] / ws4[n % 256]
    awm = (_amat() * colscale[None, :]).astype(ml_dtypes.bfloat16)
    # chunk-column-major: [128, 8*1024], chunk k at columns k*1024...
    awm = np.ascontiguousarray(
        awm.reshape(8, 128, N_FFT).transpose(1, 0, 2).reshape(128, 8 * N_FFT))

    taps = np.empty((8, 256), np.float64)
    wq = w.reshape(4, 256)
    taps[0:4] = wq / N_FFT / d4[None, :]
    sgn = 1.0 - 2.0 * (np.arange(256) % 2)
    taps[4:8] = taps[0:4] * sgn[None, :]
    taps = taps.astype(ml_dtypes.bfloat16)

    fix0 = (d4 / d3a).astype(np.float32)  # first global block: 3-frame sum
    fix7 = (d4 / d3b).astype(np.float32)  # last global block
    return awm, taps, fix0, fix7


def _inputs_for_cores(z: np.ndarray, window: np.ndarray):
    awm, taps, fix0, fix7 = _host_consts(window)

    zb = z.astype(ml_dtypes.bfloat16)
    in_maps = []
    ones = np.ones(256, np.float32)
    for c in range(N_CORES):
        G = 1024 * c - 1  # global frame index of slot 0
        X = np.zeros((1026, F_SLOTS), ml_dtypes.bfloat16)
        lo, hi = max(0, G), min(T_FRAMES, G + F_SLOTS)
        s0, s1 = lo - G, hi - G
        X[0:513, s0:s1] = zb[0, :, lo:hi]
        X[513:1024, s0:s1] = zb[1, 1:512, lo:hi]
        X[1024, s0:s1] = zb[1, 0, lo:hi]
        X[1025, s0:s1] = zb[1, 512, lo:hi]
        Xc = np.ascontiguousarray(
            X[0:1024].reshape(8, 128, F_SLOTS).transpose(1, 0, 2)
            .reshape(128, 8 * F_SLOTS))
        fix = np.stack([fix0 if c == 0 else ones,
                        fix7 if c == N_CORES - 1 else ones])
        in_maps.append({
            "x": Xc,
            "zrows": np.ascontiguousarray(X[1024:1026]),
            "awm": awm,
            "taps": taps,
            "fix": np.ascontiguousarray(fix),
        })
    return in_maps


def kernel(z: np.ndarray, window: np.ndarray) -> np.ndarray:
    from concourse.bass_utils import run_bass_kernel_spmd

    z = np.asarray(z, dtype=np.float32)
    window = np.asarray(window, dtype=np.float32)

    nc = _CACHE.get("nc")
    if nc is None:
        nc = _build_nc()
        _CACHE["nc"] = nc

    in_maps = _inputs_for_cores(z, window)
    res = run_bass_kernel_spmd(nc, in_maps, list(range(N_CORES)))

    parts = []
    for c in range(N_CORES):
        nb = NB if c < N_CORES - 1 else NB - 1
        o = np.asarray(res.results[c]["out"]).astype(np.float32)
        parts.append(o[:, :nb, :].reshape(2, -1))
    return np.ascontiguousarray(np.concatenate(parts, axis=1))


# revision 31
# speedup vs baseline: 1.0039x; 1.0039x over previous
"""Distributed ISTFT kernel for Trainium2 (8 NeuronCores, Bass/Tile).

Math (matches the jax reference):
  z: [2, 513, T] one-sided spectrum (real/imag), T = 8192 frames.
  Hermitian extension + ifft(1024) + window + overlap-add (hop 256) +
  divide by overlapped window sum + trim 512 each side -> [2, 2096896].

Key folds used here:
  * real(ifft) = A^T @ X where A [1024(k), 1024(n)] packs the cos rows for
    zr bins 0..512 and sin rows for zi bins 1..511; X packs those z rows.
  * imag(ifft)[n, t] = (zi[0,t] + (-1)^n zi[512,t]) / N  (rank-2).
  * Output sample m = 256*b + r; block b = sum_{q=0..3} wf_{b-q}[256q+r].
    Folding window * A, and the interior 1/window-sum, into the stationary
    operand on the HOST gives O^T[t, r] = sum_q X[:, t+3-q]^T @ Aw_q
    directly -- overlap-add, windowing and normalization all ride inside
    the matmul and the device never touches the window.
  * Frame axis is sharded 1024 output blocks/core with a 3-frame input
    halo, so no cross-core communication is needed at all.  The two
    blocks whose window-sum misses a frame (first and last output block)
    get a row fixup whose factor is a per-core host-computed input
    (1.0 on interior cores), so all cores run the same program.
  * Everything the PE touches is bf16 (tolerance is 2e-2; bf16 ends up
    ~3e-3): input DMA halves vs fp32, and the moving operand still
    streams at 1 column/cycle.  Output is written bf16 and widened on
    the host.
  * DMA descriptors are issued inline by the engines at ~600ns per
    dma_start, so X and Aw are stored chunk-column-major in DRAM by the
    host and fetched in 4 grouped, fully contiguous DMAs each; output
    tiles are evicted in pairs (one DMA per 256 blocks).  A short burst
    of throwaway matmuls at t=0 warms the PE's HAM clock gate
    (1.2 -> 2.4 GHz) while the first real DMAs land.
"""

import numpy as np
import ml_dtypes

N_FFT = 1024
HOP = 256
T_FRAMES = 8192
N_CORES = 8
F_SLOTS = 1027  # frame slots per core: 1024 owned blocks need slots t..t+3
NB = 1024       # output blocks computed per core (core 7 uses 1023)
N_WARM = 15     # PE warmup matmuls issued while first DMAs are in flight

_CACHE = {}


def _amat() -> np.ndarray:
    """A [1024(kappa), 1024(n)]: ifft cos/sin weights, f64."""
    n = np.arange(N_FFT, dtype=np.float64)[None, :]
    k = np.arange(513, dtype=np.float64)[:, None]
    g = np.full((513, 1), 2.0)
    g[0, 0] = 1.0
    g[512, 0] = 1.0
    C = (g / N_FFT) * np.cos(2.0 * np.pi * k * n / N_FFT)
    k2 = np.arange(1, 512, dtype=np.float64)[:, None]
    S = (-2.0 / N_FFT) * np.sin(2.0 * np.pi * k2 * n / N_FFT)
    return np.concatenate([C, S], 0)


def _build_nc():
    from contextlib import ExitStack

    import concourse.tile as tile
    from concourse import bacc, mybir

    f32 = mybir.dt.float32
    bf = mybir.dt.bfloat16

    nc = bacc.Bacc("TRN2", target_bir_lowering=False, debug=False,
                   num_devices=N_CORES)

    # x and Aw are chunk-column-major: element [p, 1027*k + s] holds
    # X[128*k + p, s], so every DMA below is a contiguous column span.
    x_d = nc.dram_tensor("x", [128, 8 * F_SLOTS], bf, kind="ExternalInput")
    z_d = nc.dram_tensor("zrows", [2, F_SLOTS], bf, kind="ExternalInput")
    a_d = nc.dram_tensor("awm", [128, 8 * N_FFT], bf, kind="ExternalInput")
    t_d = nc.dram_tensor("taps", [8, 256], bf, kind="ExternalInput")
    f_d = nc.dram_tensor("fix", [2, 256], f32, kind="ExternalInput")
    o_d = nc.dram_tensor("out", [2, NB, 256], bf, kind="ExternalOutput")

    with tile.TileContext(nc) as tc, ExitStack() as ctx:
        big = ctx.enter_context(tc.tile_pool(name="big", bufs=1))
        sml = ctx.enter_context(tc.tile_pool(name="sml", bufs=1))
        # bank budget: 6 (ps0) + 2 (transient warmup psum, then ps1) = 8
        ps0p = ctx.enter_context(tc.tile_pool(name="ps0p", bufs=6, space="PSUM"))
        osb = ctx.enter_context(tc.tile_pool(name="osb", bufs=8))

        # ---- PE warmup: dep-free matmuls on a memset tile keep the PE
        # busy from the end of the engine preamble so the HAM clock gate
        # releases (1.2 -> 2.4 GHz) while the first real DMAs land.
        wrm = sml.tile([128, 256], bf, tag="wrm")
        nc.vector.memset(wrm[:], 0.0)
        with tc.tile_pool(name="psw", bufs=1, space="PSUM") as psw:
            ps_w = psw.tile([128, 256], f32, tag="ps_warm")
            for _ in range(N_WARM):
                nc.tensor.matmul(ps_w[:], lhsT=wrm[:, 0:128], rhs=wrm[:],
                                 start=True, stop=True)
        ps1p = ctx.enter_context(tc.tile_pool(name="ps1p", bufs=2, space="PSUM"))

        # ---- big streams first, one DMA per chunk (x on the sync
        # queue, Aw on the scalar queue -- HBM is the shared limit, and
        # each queue drains FIFO in consumption order).  The first Aw
        # chunk is needed whole by matmul #1, but x chunk 0 is consumed
        # two tt-blocks at a time, so it is split into four
        # column-range tiles (262 cols each, 3-col overlap) that
        # unblock the first matmuls as soon as ~67KB have landed.
        aw_t = [None] * 8
        xs_t = [None] * 8
        for k in range(8):
            xg = big.tile([128, F_SLOTS], bf, tag=f"xg{k}", name=f"xg{k}")
            nc.sync.dma_start(
                out=xg[:], in_=x_d.ap()[:, k * F_SLOTS:(k + 1) * F_SLOTS])
            xs_t[k] = (xg, 0)
            ag = big.tile([128, N_FFT], bf, tag=f"ag{k}", name=f"ag{k}")
            nc.scalar.dma_start(
                out=ag[:], in_=a_d.ap()[:, k * N_FFT:(k + 1) * N_FFT])
            aw_t[k] = (ag, 0)

        # ---- tiny setup inputs on the gpsimd queue.  The imag-channel
        # operands are padded to the full 128 partitions with zeros so
        # the channel-1 matmuls are ordinary full-array matmuls (a
        # row-group-restricted matmul costs a ~200ns pipeline bubble
        # on entry and exit).
        taps = sml.tile([128, 256], bf, tag="taps")
        nc.vector.memset(taps[:], 0.0)
        nc.gpsimd.dma_start(out=taps[0:8, :], in_=t_d.ap())
        # partitions 0-3 = zi0 shifted by q, 4-7 = zi512 shifted, rest 0
        tuv = sml.tile([128, NB], bf, tag="tuv")
        nc.vector.memset(tuv[:], 0.0)
        for q in range(4):
            nc.gpsimd.dma_start(out=tuv[q:q + 1, :],
                                in_=z_d.ap()[0:1, 3 - q:3 - q + NB])
            nc.gpsimd.dma_start(out=tuv[4 + q:5 + q, :],
                                in_=z_d.ap()[1:2, 3 - q:3 - q + NB])
        # full-height fixup factor tiles: all-ones except the one special
        # row (engines can't address a lone partition 126; DMA can)
        fxa = sml.tile([128, 256], f32, tag="fxa")
        nc.vector.memset(fxa[:], 1.0)
        nc.gpsimd.dma_start(out=fxa[0:1, :], in_=f_d.ap()[0:1, :])
        fxb = sml.tile([128, 256], f32, tag="fxb")
        nc.vector.memset(fxb[:], 1.0)
        nc.gpsimd.dma_start(out=fxb[126:127, :], in_=f_d.ap()[1:2, :])

        def evict_half(o, half, ps, tt):
            cols = slice(256 * half, 256 * (half + 1))
            if tt == 0:
                nc.vector.tensor_mul(o[:, cols], ps[:], fxa[:])
            elif tt == 7:
                nc.vector.tensor_mul(o[:, cols], ps[:], fxb[:])
            else:
                nc.vector.tensor_copy(o[:, cols], ps[:])

        def evict_pair(ps_a, ta, ps_b, tb, ch):
            # two adjacent tt tiles -> one [128, 512] sbuf tile -> one DMA
            # covering out[ch, ta*128:(ta+2)*128, :]
            o = osb.tile([128, 512], bf, tag=f"o{ch}", name=f"o{ch}_{ta}")
            evict_half(o, 0, ps_a, ta)
            evict_half(o, 1, ps_b, tb)
            dst = o_d.ap()[ch:ch + 1, ta * 128:(tb + 1) * 128, :]
            nc.sync.dma_start(
                out=dst.rearrange("a (h p) r -> a p h r", p=128), in_=o[:])

        def evict_one(ps, tt, ch):
            o = osb.tile([128, 256], bf, tag=f"s{ch}", name=f"s{ch}_{tt}")
            evict_half(o, 0, ps, tt)
            nc.sync.dma_start(
                out=o_d.ap()[ch:ch + 1, tt * 128:(tt + 1) * 128, :], in_=o[:])
            return o

        def ch1_pair(ta):
            # both imag-channel blocks of a pair back-to-back (one
            # row-group switch), then one paired eviction DMA
            ps = []
            for tt in (ta, ta + 1):
                p1 = ps1p.tile([128, 256], f32, tag="ps1", name=f"ps1_{tt}")
                nc.tensor.matmul(p1[:], lhsT=tuv[:, tt * 128:tt * 128 + 128],
                                 rhs=taps[:], start=True, stop=True)
                ps.append(p1)
            evict_pair(ps[0], ta, ps[1], ta + 1, 1)

        def mm_block(ps, tt, k, qs):
            xg, xb = xs_t[k]
            ag, ab = aw_t[k]
            for q in qs:
                off = xb + tt * 128 + 3 - q
                nc.tensor.matmul(
                    ps[:],
                    lhsT=xg[:, off:off + 128],
                    rhs=ag[:, ab + 256 * q:ab + 256 * (q + 1)],
                    start=(k == 0 and q == 0),
                    stop=(k == 7 and q == 3))

        # ---- channel 0 sweep 1: k-outer accumulation over six psum
        # tiles, following the arriving k chunks; paired evictions
        pss = {tt: ps0p.tile([128, 256], f32, tag="ps0", name=f"ps0_{tt}")
               for tt in range(6)}
        for k in range(8):
            for tt in range(6):
                mm_block(pss[tt], tt, k, range(4))
        for ta in (0, 2, 4):
            evict_pair(pss[ta], ta, pss[ta + 1], ta + 1, 0)

        # ---- sweep 2: tt-outer (all inputs are resident by now) so the
        # tail holds only one eviction; channel-1 pairs slot between
        # half-blocks, away from the very end
        ps6 = ps0p.tile([128, 256], f32, tag="ps0", name="ps0_6")
        for k in range(4):
            mm_block(ps6, 6, k, range(4))
        ch1_pair(0)
        for k in range(4, 8):
            mm_block(ps6, 6, k, range(4))
        ch1_pair(2)
        s06 = evict_one(ps6, 6, 0)
        ps7 = ps0p.tile([128, 256], f32, tag="ps0", name="ps0_7")
        for k in range(2):
            mm_block(ps7, 7, k, range(4))
        ch1_pair(4)
        for k in range(2, 4):
            mm_block(ps7, 7, k, range(4))
        ch1_pair(6)
        for k in range(4, 7):
            mm_block(ps7, 7, k, range(4))
        # re-write 4KB of the already-evicted tt6 tile to keep the sync
        # DMA queue's engines hot, so the final eviction's DMA doesn't
        # pay a cold-queue kick right on the critical tail
        nc.sync.dma_start(
            out=o_d.ap()[0:1, 6 * 128:6 * 128 + 8, :], in_=s06[0:8, :])
        mm_block(ps7, 7, 7, range(4))
        evict_one(ps7, 7, 0)

    nc.compile()
    return nc


def _host_consts(window: np.ndarray):
    """Window-dependent host folds: Aw matrix, ch1 taps, edge fixups."""
    w = np.asarray(window, np.float64)
    ws4 = w[0:256] + w[256:512] + w[512:768] + w[768:1024]
    # reference guard: only divide where the overlapped window sum >= 1e-6
    d4 = np.where(ws4 >= 1e-6, ws4, 1.0)
    d3a = np.where(ws4 - w[768:1024] >= 1e-6, ws4 - w[768:1024], 1.0)
    d3b = np.where(ws4 - w[0:256] >= 1e-6, ws4 - w[0:256], 1.0)

    colscale = w / np.tile(d4, 4)                   # w[n] / ws4[n % 256]
    awm = (_amat() * colscale[None, :]).astype(ml_dtypes.bfloat16)
    # chunk-column-major: [128, 8*1024], chunk k at columns k*1024...
    awm = np.ascontiguousarray(
        awm.reshape(8, 128, N_FFT).transpose(1, 0, 2).reshape(128, 8 * N_FFT))

    taps = np.empty((8, 256), np.float64)
    wq = w.reshape(4, 256)
    taps[0:4] = wq / N_FFT / d4[None, :]
    sgn = 1.0 - 2.0 * (np.arange(256) % 2)
    taps[4:8] = taps[0:4] * sgn[None, :]
    taps = taps.astype(ml_dtypes.bfloat16)

    fix0 = (d4 / d3a).astype(np.float32)  # first global block: 3-frame sum
    fix7 = (d4 / d3b).astype(np.float32)  # last global block
    return awm, taps, fix0, fix7


def _inputs_for_cores(z: np.ndarray, window: np.ndarray):
    awm, taps, fix0, fix7 = _host_consts(window)

    zb = z.astype(ml_dtypes.bfloat16)
    in_maps = []
    ones = np.ones(256, np.float32)
    for c in range(N_CORES):
        G = 1024 * c - 1  # global frame index of slot 0
        X = np.zeros((1026, F_SLOTS), ml_dtypes.bfloat16)
        lo, hi = max(0, G), min(T_FRAMES, G + F_SLOTS)
        s0, s1 = lo - G, hi - G
        X[0:513, s0:s1] = zb[0, :, lo:hi]
        X[513:1024, s0:s1] = zb[1, 1:512, lo:hi]
        X[1024, s0:s1] = zb[1, 0, lo:hi]
        X[1025, s0:s1] = zb[1, 512, lo:hi]
        Xc = np.ascontiguousarray(
            X[0:1024].reshape(8, 128, F_SLOTS).transpose(1, 0, 2)
            .reshape(128, 8 * F_SLOTS))
        fix = np.stack([fix0 if c == 0 else ones,
                        fix7 if c == N_CORES - 1 else ones])
        in_maps.append({
            "x": Xc,
            "zrows": np.ascontiguousarray(X[1024:1026]),
            "awm": awm,
            "taps": taps,
            "fix": np.ascontiguousarray(fix),
        })
    return in_maps


def kernel(z: np.ndarray, window: np.ndarray) -> np.ndarray:
    from concourse.bass_utils import run_bass_kernel_spmd

    z = np.asarray(z, dtype=np.float32)
    window = np.asarray(window, dtype=np.float32)

    nc = _CACHE.get("nc")
    if nc is None:
        nc = _build_nc()
        _CACHE["nc"] = nc

    in_maps = _inputs_for_cores(z, window)
    res = run_bass_kernel_spmd(nc, in_maps, list(range(N_CORES)))

    parts = []
    for c in range(N_CORES):
        nb = NB if c < N_CORES - 1 else NB - 1
        o = np.asarray(res.results[c]["out"]).astype(np.float32)
        parts.append(o[:, :nb, :].reshape(2, -1))
    return np.ascontiguousarray(np.concatenate(parts, axis=1))
